# revision 2
# baseline (speedup 1.0000x reference)
"""Trainium2 Bass kernel for nn_CustomFullyConnectedLayerGoogleTopK (v8).

Math (see derivation in prepare_in_maps):
    out[b, r] = sum_c x[b, c] * V[(r-c)%N, c] * a[(r-c)%N]
    a = min(K/sum(exp(alpha)) * exp(alpha), 1)
Output columns sharded 8 ways; contraction rows presented reversed so the
scale field is the ascending Toeplitz window win[p, i] = alpha2[1 + p + i].

Schedule notes (evolved over v2-v5 traces):
  * <= ~10 live DMA-completion semaphore lanes: Tile recycles them
    round-robin and DMA #n+L blocks issue until #n completes, so the issue
    order keeps every recycled lane on an early-completing predecessor.
  * tiny alpha input on the otherwise idle gpsimd ring -> the softmax
    normalizer c = K * reciprocal(sum exp) is ready by ~12 us; the window is
    split so chunk 0's exp runs as soon as the first piece lands. First real
    matmul ~14 us, right behind the warm-up+keep-alive matmuls (HAM stays
    at 2.4 GHz).
  * scalar ring: win_a, x[8:20), win_b, x[20:32); sync ring: x[0:8) + band
    batches [8,8,8,4,4] blocks - consumption order, descriptors >= 4 KB.
  * window chunks exp'd on Scalar, fused (*c, min 1) on Vector; the band
    multiply runs in place with sub-batches of <= 4 blocks, alternating
    between Vector and GpSimd so the PE never waits on the multiply engine.
  * output bf16 through a uint16-typed tensor, both halves copied+stored on
    different engines in parallel.
"""

import math
import os
import sys

import numpy as np

for _p in ("/opt/trn_rl_repo", "/root/.axon_site/_ro/trn_rl_repo"):
    if os.path.isdir(_p) and _p not in sys.path:
        sys.path.append(_p)

import ml_dtypes

import concourse.bacc as bacc
import concourse.bass as bass
import concourse.mybir as mybir
import concourse.tile as tile
from concourse.bass_utils import run_bass_kernel_spmd

F32 = mybir.dt.float32
BF16 = mybir.dt.bfloat16
U16 = mybir.dt.uint16
BF16_NP = ml_dtypes.bfloat16

N = 4096          # IN_F == OUT_F == N_PERM == DIAG
B = 256           # batch
NCORES = 8
RW = N // NCORES  # 512 output columns per core
K_TOPK = 3687     # ceil(0.9 * 4096 * 4096 / 4096)
CB = 128          # contraction block (SBUF partition count)
NCB = N // CB     # 32 contraction blocks
WIN_W = RW + (NCB - 1) * CB  # 4480

BATCHES = [(0, 8), (8, 16), (16, 24), (24, 28), (28, 32)]
SUBS = [(0, 2), (2, 4), (4, 8), (8, 12), (12, 16), (16, 20), (20, 24),
        (24, 28), (28, 30), (30, 32)]
CHUNK_ENDS = [CB * (k1 - 1) + RW for (_, k1) in SUBS]
WINPIECES = [(0, 1408), (1408, 2944), (2944, WIN_W)]  # align with CHUNK_ENDS
XT_PIECES = [(0, 8), (8, 16), (16, 24), (24, 32)]


def _cols(ap2d, col_off, shape_strides):
    pstep = ap2d.ap[0][0]
    return bass.AP(
        ap2d.tensor, ap2d.offset + col_off,
        [[pstep, 128]] + shape_strides,
    )


def _build_program():
    nc = bacc.Bacc("TRN2", target_bir_lowering=False, debug=False)

    band16 = nc.dram_tensor("band16", [CB, NCB * RW], BF16, kind="ExternalInput").ap()
    xt16 = nc.dram_tensor("xt16", [CB, NCB * B], BF16, kind="ExternalInput").ap()
    win16 = nc.dram_tensor("win16", [CB, WIN_W], BF16, kind="ExternalInput").ap()
    alpha32 = nc.dram_tensor("alpha32", [N], F32, kind="ExternalInput").ap()
    out = nc.dram_tensor("out", [B, RW], U16, kind="ExternalOutput").ap()

    with tile.TileContext(nc) as tc:
        with (
            tc.tile_pool(name="small", bufs=1) as sp,
            tc.tile_pool(name="winp", bufs=1) as wp,
            tc.tile_pool(name="vb", bufs=1) as vbp,
            tc.tile_pool(name="xtp", bufs=1) as xtp,
            tc.tile_pool(name="opool", bufs=1) as op,
            tc.tile_pool(name="psum", bufs=1, space="PSUM") as pp,
            tc.tile_pool(name="psum_s", bufs=1, space="PSUM") as pps,
        ):
            # ---- DMA issue: ONE ring (sync) in exact consumption order.
            # A single queue gets all 16 SDMA engines (no per-packet share
            # dilution across queues) and same-ring semaphore-lane recycling
            # can never stall (FIFO). alpha rides SWDGE on the idle gpsimd.
            alpha_sb = sp.tile([128, N // 128], F32)
            nc.gpsimd.dma_start(
                alpha_sb[:], alpha32.rearrange("(p f) -> p f", p=128)
            )

            xt = xtp.tile([128, NCB, B], BF16)

            def xt_dma(lo, hi):
                nc.sync.dma_start(
                    xt[:, lo:hi, :], xt16[:, lo * B:hi * B],
                )

            win = wp.tile([128, WIN_W], BF16)

            def win_dma(lo, hi):
                nc.sync.dma_start(
                    _cols(win[:], lo, [[1, hi - lo]]), win16[:, lo:hi]
                )

            vb = {}

            def band_dma(k0, k1):
                vbq = vbp.tile([128, (k1 - k0) * RW], BF16, tag=f"vb{k0}")
                nc.sync.dma_start(vbq[:], band16[:, k0 * RW:k1 * RW])
                vb[k0] = vbq

            win_dma(*WINPIECES[0])
            xt_dma(*XT_PIECES[0])
            band_dma(*BATCHES[0])
            xt_dma(*XT_PIECES[1])
            win_dma(*WINPIECES[1])
            band_dma(*BATCHES[1])
            xt_dma(*XT_PIECES[2])
            band_dma(*BATCHES[2])
            win_dma(*WINPIECES[2])
            xt_dma(*XT_PIECES[3])
            band_dma(*BATCHES[3])
            band_dma(*BATCHES[4])

            # ---- PE warm-up (no data deps; bridges HAM window) ----
            ones16 = sp.tile([128, 128], BF16)
            nc.vector.memset(ones16[:], 1.0)
            warm_rhs = sp.tile([128, RW], BF16)
            nc.vector.memset(warm_rhs[:], 0.0)
            warm_a = pps.tile([128, RW], F32)
            warm_b = pps.tile([128, RW], F32)

            def warm_mm(i):
                dst = warm_a if i % 2 == 0 else warm_b
                nc.tensor.matmul(dst[:], ones16[:], warm_rhs[:],
                                 start=True, stop=True)

            for i in range(12):
                warm_mm(i)

            # ---- c = K / sum(exp(alpha)), per-partition copies ----
            exp_sb = sp.tile([128, N // 128], F32)
            rowsum = sp.tile([128, 1], F32)
            nc.scalar.activation(
                exp_sb[:], alpha_sb[:], mybir.ActivationFunctionType.Exp,
                accum_out=rowsum[:],
            )
            ones = sp.tile([128, 128], F32)
            nc.vector.memset(ones[:], 1.0)
            tot_ps = pps.tile([128, 1], F32)
            nc.tensor.matmul(tot_ps[:], ones[:], rowsum[:], start=True, stop=True)
            for i in range(6):
                warm_mm(i)
            recip = sp.tile([128, 1], F32)
            nc.vector.reciprocal(recip[:], tot_ps[:])
            c_sc = sp.tile([128, 1], F32)
            nc.vector.tensor_scalar_mul(c_sc[:], recip[:], float(K_TOPK))

            # ---- window chunks: exp on Scalar, fused (*c, min 1) on Vector
            def process_chunk(ci):
                lo = CHUNK_ENDS[ci - 1] if ci > 0 else 0
                hi = CHUNK_ENDS[ci]
                view = _cols(win[:], lo, [[1, hi - lo]])
                nc.scalar.activation(
                    view, view, mybir.ActivationFunctionType.Exp,
                )
                nc.vector.tensor_scalar(
                    view, view, c_sc[:, 0:1], 1.0,
                    mybir.AluOpType.mult, mybir.AluOpType.min,
                )

            # ---- main pipeline; multiplies alternate Vector / GpSimd ----
            psum0 = pp.tile([128, RW], F32)
            psum1 = pp.tile([128, RW], F32)
            for si, (k0, k1) in enumerate(SUBS):
                process_chunk(si)
                nk = k1 - k0
                b0 = max(b for (b, _) in BATCHES if b <= k0)
                view = _cols(vb[b0][:], (k0 - b0) * RW, [[RW, nk], [1, RW]])
                nc.vector.tensor_tensor(
                    view, view,
                    _cols(win[:], k0 * CB, [[CB, nk], [1, RW]]),
                    mybir.AluOpType.mult,
                )
                for t in range(nk):
                    k = k0 + t
                    wt_k = _cols(vb[b0][:], (k - b0) * RW, [[1, RW]])
                    nc.tensor.matmul(psum0[:], xt[:, k, 0:128], wt_k,
                                     start=(k == 0), stop=(k == NCB - 1))
                    nc.tensor.matmul(psum1[:], xt[:, k, 128:256], wt_k,
                                     start=(k == 0), stop=(k == NCB - 1))

            # ---- PSUM -> SBUF (bf16) -> DRAM as uint16, two engines ----
            o0 = op.tile([128, RW], BF16)
            nc.scalar.activation(o0[:], psum0[:], mybir.ActivationFunctionType.Copy)
            nc.scalar.dma_start(out[0:128, :], o0[:].bitcast(U16))
            o1 = op.tile([128, RW], BF16)
            nc.vector.tensor_copy(o1[:], psum1[:])
            nc.sync.dma_start(out[128:256, :], o1[:].bitcast(U16))

    nc.compile()
    return nc


_NC_CACHE = []


def _get_program():
    if not _NC_CACHE:
        _NC_CACHE.append(_build_program())
    return _NC_CACHE[0]


def prepare_in_maps(x: np.ndarray, V: np.ndarray, alpha: np.ndarray):
    """Shard + lay out the full inputs into 8 per-core input maps (bf16).

    Device contraction row p of block k holds c = N-1-(128k+p); then
    scale[p, k, j] = a[(r0+1+128k+p+j)%N] = win[p, 128k+j] with
    win[p, i] = a2[1+p+i], a2 = doubled roll(alpha, -r0), and
    band[p, k, j] = V[(r0+1+128k+p+j)%N, N-1-128k-p].
    """
    x = np.ascontiguousarray(np.asarray(x, dtype=np.float32))
    V = np.ascontiguousarray(np.asarray(V, dtype=np.float32))
    alpha = np.ascontiguousarray(np.asarray(alpha, dtype=np.float32))

    xT = np.ascontiguousarray(x.T[::-1, :])
    xt_host = np.ascontiguousarray(
        xT.reshape(NCB, CB, B).transpose(1, 0, 2).astype(BF16_NP)
    ).reshape(CB, NCB * B)

    Vt = np.ascontiguousarray(V.T)
    VtD = np.concatenate([Vt, Vt], axis=1)  # (N, 2N)
    flat = VtD.reshape(-1)
    isz = flat.itemsize

    in_maps = []
    for m in range(NCORES):
        r0 = m * RW
        start = N + r0
        band_bl = np.lib.stride_tricks.as_strided(
            flat[start:], shape=(N, RW), strides=((2 * N - 1) * isz, isz),
        )[::-1]
        band_host = np.ascontiguousarray(
            band_bl.reshape(NCB, CB, RW).transpose(1, 0, 2).astype(BF16_NP)
        ).reshape(CB, NCB * RW)

        am = np.roll(alpha, -r0)
        a2 = np.concatenate([am, am]).astype(BF16_NP)
        win_host = np.ascontiguousarray(np.lib.stride_tricks.as_strided(
            a2[1:], shape=(CB, WIN_W), strides=(2, 2),
        ))

        in_maps.append({
            "band16": band_host,
            "xt16": xt_host,
            "win16": win_host,
            "alpha32": alpha,
        })
    return in_maps


def out_to_f32(arr: np.ndarray) -> np.ndarray:
    """uint16-typed device output -> bf16 bits -> float32."""
    return np.asarray(arr).view(BF16_NP).astype(np.float32)


def gather_output(results) -> np.ndarray:
    return np.concatenate(
        [out_to_f32(results[m]["out"]) for m in range(NCORES)], axis=1
    )


def kernel(x: np.ndarray, V: np.ndarray, alpha: np.ndarray) -> np.ndarray:
    in_maps = prepare_in_maps(x, V, alpha)
    nc = _get_program()
    res = run_bass_kernel_spmd(nc, in_maps, core_ids=list(range(NCORES)))
    return gather_output(res.results)


# revision 3
# speedup vs baseline: 1.0272x; 1.0272x over previous
"""Trainium2 Bass kernel for nn_CustomFullyConnectedLayerGoogleTopK (v8).

Math (see derivation in prepare_in_maps):
    out[b, r] = sum_c x[b, c] * V[(r-c)%N, c] * a[(r-c)%N]
    a = min(K/sum(exp(alpha)) * exp(alpha), 1)
Output columns sharded 8 ways; contraction rows presented reversed so the
scale field is the ascending Toeplitz window win[p, i] = alpha2[1 + p + i].

Schedule notes (from perfetto/ntff trace iterations):
  * all bulk inputs ride ONE HWDGE ring (sync) in exact consumption order:
    win[0:1408], x[0:8), band[0:8), x[8:16), win[1408:2944], band[8:16),
    x[16:24), band[16:24), win[2944:], x[24:32), band[24:28), band[28:32).
    A single queue gets all 16 SDMA engines (competing queues dilute each
    other per-packet and do NOT sum), and same-ring completion-semaphore
    lane recycling can never stall the issue stream.
  * tiny alpha input on the otherwise idle gpsimd (SWDGE) ring -> the
    softmax normalizer c = K * reciprocal(sum exp) is ready early; window
    chunks are exp'd on Scalar as pieces land, then fused (*c, min 1) on
    Vector. No Ln (avoids activation-table thrash).
  * the band multiply runs in place (band tile *= scale view) on Vector in
    sub-batches of <= 4 blocks feeding the PE matmul stream.
  * 12 warm-up + 6 keep-alive matmuls bridge the HAM activity window so the
    GEMM runs at 2.4 GHz when real data arrives.
  * output bf16 through a uint16-typed tensor (the PJRT path rejects bf16
    outputs), both halves copied+stored on different engines in parallel.
"""

import math
import os
import sys

import numpy as np

for _p in ("/opt/trn_rl_repo", "/root/.axon_site/_ro/trn_rl_repo"):
    if os.path.isdir(_p) and _p not in sys.path:
        sys.path.append(_p)

import ml_dtypes

import concourse.bacc as bacc
import concourse.bass as bass
import concourse.mybir as mybir
import concourse.tile as tile
from concourse.bass_utils import run_bass_kernel_spmd

F32 = mybir.dt.float32
BF16 = mybir.dt.bfloat16
U16 = mybir.dt.uint16
BF16_NP = ml_dtypes.bfloat16

N = 4096          # IN_F == OUT_F == N_PERM == DIAG
B = 256           # batch
NCORES = 8
RW = N // NCORES  # 512 output columns per core
K_TOPK = 3687     # ceil(0.9 * 4096 * 4096 / 4096)
CB = 128          # contraction block (SBUF partition count)
NCB = N // CB     # 32 contraction blocks
WIN_W = RW + (NCB - 1) * CB  # 4480

BATCHES = [(0, 8), (8, 16), (16, 24), (24, 28), (28, 32)]
SUBS = [(0, 2), (2, 4), (4, 8), (8, 12), (12, 16), (16, 20), (20, 24),
        (24, 28), (28, 30), (30, 32)]
CHUNK_ENDS = [CB * (k1 - 1) + RW for (_, k1) in SUBS]
WINPIECES = [(0, 1408), (1408, 2944), (2944, WIN_W)]  # align with CHUNK_ENDS
XT_PIECES = [(0, 8), (8, 16), (16, 24), (24, 32)]


def _cols(ap2d, col_off, shape_strides):
    pstep = ap2d.ap[0][0]
    return bass.AP(
        ap2d.tensor, ap2d.offset + col_off,
        [[pstep, 128]] + shape_strides,
    )


def _build_program():
    nc = bacc.Bacc("TRN2", target_bir_lowering=False, debug=False)

    band16 = nc.dram_tensor("band16", [CB, NCB * RW], BF16, kind="ExternalInput").ap()
    xt16 = nc.dram_tensor("xt16", [CB, NCB * B], BF16, kind="ExternalInput").ap()
    win16 = nc.dram_tensor("win16", [CB, WIN_W], BF16, kind="ExternalInput").ap()
    alpha32 = nc.dram_tensor("alpha32", [N], F32, kind="ExternalInput").ap()
    out = nc.dram_tensor("out", [B, RW], U16, kind="ExternalOutput").ap()

    with tile.TileContext(nc) as tc:
        with (
            tc.tile_pool(name="small", bufs=1) as sp,
            tc.tile_pool(name="winp", bufs=1) as wp,
            tc.tile_pool(name="vb", bufs=1) as vbp,
            tc.tile_pool(name="xtp", bufs=1) as xtp,
            tc.tile_pool(name="opool", bufs=1) as op,
            tc.tile_pool(name="psum", bufs=1, space="PSUM") as pp,
            tc.tile_pool(name="psum_s", bufs=1, space="PSUM") as pps,
        ):
            # ---- DMA issue: ONE ring (sync) in exact consumption order.
            # A single queue gets all 16 SDMA engines (no per-packet share
            # dilution across queues) and same-ring semaphore-lane recycling
            # can never stall (FIFO). alpha rides SWDGE on the idle gpsimd.
            alpha_sb = sp.tile([128, N // 128], F32)
            nc.gpsimd.dma_start(
                alpha_sb[:], alpha32.rearrange("(p f) -> p f", p=128)
            )

            xt = xtp.tile([128, NCB, B], BF16)

            def xt_dma(lo, hi):
                nc.sync.dma_start(
                    xt[:, lo:hi, :], xt16[:, lo * B:hi * B],
                )

            win = wp.tile([128, WIN_W], BF16)

            def win_dma(lo, hi):
                nc.sync.dma_start(
                    _cols(win[:], lo, [[1, hi - lo]]), win16[:, lo:hi]
                )

            vb = {}

            def band_dma(k0, k1):
                vbq = vbp.tile([128, (k1 - k0) * RW], BF16, tag=f"vb{k0}")
                nc.sync.dma_start(vbq[:], band16[:, k0 * RW:k1 * RW])
                vb[k0] = vbq

            win_dma(*WINPIECES[0])
            xt_dma(*XT_PIECES[0])
            band_dma(*BATCHES[0])
            xt_dma(*XT_PIECES[1])
            win_dma(*WINPIECES[1])
            band_dma(*BATCHES[1])
            xt_dma(*XT_PIECES[2])
            band_dma(*BATCHES[2])
            win_dma(*WINPIECES[2])
            xt_dma(*XT_PIECES[3])
            band_dma(*BATCHES[3])
            band_dma(*BATCHES[4])

            # ---- PE warm-up (no data deps; bridges HAM window) ----
            ones16 = sp.tile([128, 128], BF16)
            nc.vector.memset(ones16[:], 1.0)
            warm_rhs = sp.tile([128, RW], BF16)
            nc.vector.memset(warm_rhs[:], 0.0)
            warm_a = pps.tile([128, RW], F32)
            warm_b = pps.tile([128, RW], F32)

            def warm_mm(i):
                dst = warm_a if i % 2 == 0 else warm_b
                nc.tensor.matmul(dst[:], ones16[:], warm_rhs[:],
                                 start=True, stop=True)

            for i in range(12):
                warm_mm(i)

            # ---- c = K / sum(exp(alpha)), per-partition copies ----
            exp_sb = sp.tile([128, N // 128], F32)
            rowsum = sp.tile([128, 1], F32)
            nc.scalar.activation(
                exp_sb[:], alpha_sb[:], mybir.ActivationFunctionType.Exp,
                accum_out=rowsum[:],
            )
            ones = sp.tile([128, 128], F32)
            nc.vector.memset(ones[:], 1.0)
            tot_ps = pps.tile([128, 1], F32)
            nc.tensor.matmul(tot_ps[:], ones[:], rowsum[:], start=True, stop=True)
            for i in range(6):
                warm_mm(i)
            recip = sp.tile([128, 1], F32)
            nc.vector.reciprocal(recip[:], tot_ps[:])
            c_sc = sp.tile([128, 1], F32)
            nc.vector.tensor_scalar_mul(c_sc[:], recip[:], float(K_TOPK))

            # ---- window chunks: exp on Scalar, fused (*c, min 1) on Vector
            def process_chunk(ci):
                lo = CHUNK_ENDS[ci - 1] if ci > 0 else 0
                hi = CHUNK_ENDS[ci]
                view = _cols(win[:], lo, [[1, hi - lo]])
                nc.scalar.activation(
                    view, view, mybir.ActivationFunctionType.Exp,
                )
                nc.vector.tensor_scalar(
                    view, view, c_sc[:, 0:1], 1.0,
                    mybir.AluOpType.mult, mybir.AluOpType.min,
                )

            # ---- main pipeline; multiplies alternate Vector / GpSimd ----
            psum0 = pp.tile([128, RW], F32)
            psum1 = pp.tile([128, RW], F32)
            for si, (k0, k1) in enumerate(SUBS):
                process_chunk(si)
                nk = k1 - k0
                b0 = max(b for (b, _) in BATCHES if b <= k0)
                view = _cols(vb[b0][:], (k0 - b0) * RW, [[RW, nk], [1, RW]])
                nc.vector.tensor_tensor(
                    view, view,
                    _cols(win[:], k0 * CB, [[CB, nk], [1, RW]]),
                    mybir.AluOpType.mult,
                )
                for t in range(nk):
                    k = k0 + t
                    wt_k = _cols(vb[b0][:], (k - b0) * RW, [[1, RW]])
                    nc.tensor.matmul(psum0[:], xt[:, k, 0:128], wt_k,
                                     start=(k == 0), stop=(k == NCB - 1))
                    nc.tensor.matmul(psum1[:], xt[:, k, 128:256], wt_k,
                                     start=(k == 0), stop=(k == NCB - 1))

            # ---- PSUM -> SBUF (bf16) -> DRAM as uint16, two engines ----
            o0 = op.tile([128, RW], BF16)
            nc.scalar.activation(o0[:], psum0[:], mybir.ActivationFunctionType.Copy)
            nc.scalar.dma_start(out[0:128, :], o0[:].bitcast(U16))
            o1 = op.tile([128, RW], BF16)
            nc.vector.tensor_copy(o1[:], psum1[:])
            nc.sync.dma_start(out[128:256, :], o1[:].bitcast(U16))

    nc.compile()
    return nc


_NC_CACHE = []


def _get_program():
    if not _NC_CACHE:
        _NC_CACHE.append(_build_program())
    return _NC_CACHE[0]


def prepare_in_maps(x: np.ndarray, V: np.ndarray, alpha: np.ndarray):
    """Shard + lay out the full inputs into 8 per-core input maps (bf16).

    Device contraction row p of block k holds c = N-1-(128k+p); then
    scale[p, k, j] = a[(r0+1+128k+p+j)%N] = win[p, 128k+j] with
    win[p, i] = a2[1+p+i], a2 = doubled roll(alpha, -r0), and
    band[p, k, j] = V[(r0+1+128k+p+j)%N, N-1-128k-p].
    """
    x = np.ascontiguousarray(np.asarray(x, dtype=np.float32))
    V = np.ascontiguousarray(np.asarray(V, dtype=np.float32))
    alpha = np.ascontiguousarray(np.asarray(alpha, dtype=np.float32))

    xT = np.ascontiguousarray(x.T[::-1, :])
    xt_host = np.ascontiguousarray(
        xT.reshape(NCB, CB, B).transpose(1, 0, 2).astype(BF16_NP)
    ).reshape(CB, NCB * B)

    Vt = np.ascontiguousarray(V.T)
    VtD = np.concatenate([Vt, Vt], axis=1)  # (N, 2N)
    flat = VtD.reshape(-1)
    isz = flat.itemsize

    in_maps = []
    for m in range(NCORES):
        r0 = m * RW
        start = N + r0
        band_bl = np.lib.stride_tricks.as_strided(
            flat[start:], shape=(N, RW), strides=((2 * N - 1) * isz, isz),
        )[::-1]
        band_host = np.ascontiguousarray(
            band_bl.reshape(NCB, CB, RW).transpose(1, 0, 2).astype(BF16_NP)
        ).reshape(CB, NCB * RW)

        am = np.roll(alpha, -r0)
        a2 = np.concatenate([am, am]).astype(BF16_NP)
        win_host = np.ascontiguousarray(np.lib.stride_tricks.as_strided(
            a2[1:], shape=(CB, WIN_W), strides=(2, 2),
        ))

        in_maps.append({
            "band16": band_host,
            "xt16": xt_host,
            "win16": win_host,
            "alpha32": alpha,
        })
    return in_maps


def out_to_f32(arr: np.ndarray) -> np.ndarray:
    """uint16-typed device output -> bf16 bits -> float32."""
    return np.asarray(arr).view(BF16_NP).astype(np.float32)


def gather_output(results) -> np.ndarray:
    return np.concatenate(
        [out_to_f32(results[m]["out"]) for m in range(NCORES)], axis=1
    )


def kernel(x: np.ndarray, V: np.ndarray, alpha: np.ndarray) -> np.ndarray:
    in_maps = prepare_in_maps(x, V, alpha)
    nc = _get_program()
    res = run_bass_kernel_spmd(nc, in_maps, core_ids=list(range(NCORES)))
    return gather_output(res.results)
